# revision 1
# baseline (speedup 1.0000x reference)
"""BEV pooling (LSS view transform) kernel for Trainium2, 8 NeuronCores.

Problem: x (B=4, D=118, H=32, W=88, C=80) camera frustum features are pooled
into a (B, C, 360, 360) BEV grid via voxel scatter-add (segment_sum).

Structure exploited (verified at runtime from the actual inputs):
  - camera->lidar maps pixel (u, v, d): BEV voxel depends on (d, w) only and
    the z-range keep-mask on (d, h) only.
  - So  pooled[vox(d,w)] += sum_h zmask(d,h) * x[d,h,w,:], and within a d-row
    equal-voxel groups are consecutive runs in w.

Device kernel per core (core = one batch x one 44-column w-half; runs that
cross the w boundary give partial sums in each core's private grid, which the
host adds):
  Stage A: stream x in bf16 [128, 3520] tiles, laid out (d h)(c w); PE bf16
           matmul with a block-diagonal 0/1 h-mask reduces over h into one
           PSUM tile y[118, (c w)] (d<64 at quadrant 0, d>=64 at quadrant 64,
           so no mid-stream PSUM copy is needed).
  Stage B: ONE DVE tensor_tensor_scan (state = m*state + y, fp32 state) along
           the w-innermost free dim computes every run's total at its run-END
           slot; then one strided tensor_copy transposes (c w) -> (w c).
  Stage C: dma_scatter_add (the SWDGE extended instruction, ~0.34ns/desc) in
           prepare_only mode, one call per 32768-row grid window (int16 index
           limit). Preps generate descriptors early (their y_t read defers to
           the trigger); non-run-end / out-of-range tokens aim at an unused
           trash row inside their window.

The grid DRAM tensor is pre-zeroed by the runner (documented contract of
run_bass_kernel_spmd), so untouched voxels read 0 and a single scatter-ADD
per real voxel equals a plain write.
"""

import os
import sys

import numpy as np

sys.path.insert(0, "/opt/trn_rl_repo")

# ---- problem constants (hardcoded per spec) ----
B, D, H, W, C = 4, 118, 32, 88, 80
WS = W // 2  # per-core w-column span (cores shard on batch x w-half)
NXX = NXY = 360
NZ = 1
V = NXX * NXY  # voxels per batch slice
DX = np.array([0.3, 0.3, 20.0], np.float32)
BX_LO = np.array([-54.0, -54.0, -10.0], np.float32)
N_CORES = 8
GROUPS = (D + 3) // 4  # 30 groups of <=4 d-slabs
WC = WS * C  # 3520

WINR = 32400  # real grid rows per scatter window (V = 4 * WINR)
WINP = 32768  # padded rows per window (int16 index space)
TRASH = WINR  # in-window row for discarded tokens (rows WINR..WINP-1 spare)
NTOK = WS * 128  # scatter tokens per call (44 w-slots x 128 partitions)
SG = 15  # 8-d super-groups per core (d padded 118 -> 120)

_NC_CACHE: dict = {}


def _host_coords(x, camera2lidar_rots, camera2lidar_trans, intrins, frustum):
    """Voxel int coords for every point, bit-identical to the reference
    (same jax ops on the cpu backend)."""
    import jax
    import jax.numpy as jnp

    cpu = jax.devices("cpu")[0]
    with jax.default_device(cpu):
        frustum = jnp.asarray(np.asarray(frustum))
        rots = jnp.asarray(np.asarray(camera2lidar_rots))
        trans = jnp.asarray(np.asarray(camera2lidar_trans))
        intr = jnp.asarray(np.asarray(intrins))
        pts = jnp.concatenate(
            [frustum[..., :2] * frustum[..., 2:3], frustum[..., 2:3]], axis=-1
        )
        combine = rots @ jnp.linalg.inv(intr)
        geom = (
            jnp.einsum("bij,dhwj->bdhwi", combine, pts)
            + trans[:, None, None, None, :]
        )
        coords = ((geom - jnp.asarray(BX_LO)) / jnp.asarray(DX)).astype(jnp.int32)
        coords = np.asarray(jax.device_get(coords))
    return coords  # (B, D, H, W, 3) int32


def _host_fallback(x, camera2lidar_rots, camera2lidar_trans, intrins, frustum):
    """Exact reference computation on host (jax cpu). Correct for arbitrary
    inputs; used only if the factorized structure doesn't hold."""
    import jax
    import jax.numpy as jnp

    cpu = jax.devices("cpu")[0]
    with jax.default_device(cpu):
        x = jnp.asarray(np.asarray(x))
        rots = jnp.asarray(np.asarray(camera2lidar_rots))
        trans = jnp.asarray(np.asarray(camera2lidar_trans))
        intr = jnp.asarray(np.asarray(intrins))
        frustum = jnp.asarray(np.asarray(frustum))
        b, d, h, w, c = x.shape
        pts = jnp.concatenate(
            [frustum[..., :2] * frustum[..., 2:3], frustum[..., 2:3]], axis=-1
        )
        combine = rots @ jnp.linalg.inv(intr)
        geom = (
            jnp.einsum("bij,dhwj->bdhwi", combine, pts)
            + trans[:, None, None, None, :]
        )
        feats = x.reshape(-1, c)
        coords = ((geom - jnp.asarray(BX_LO)) / jnp.asarray(DX)).astype(
            jnp.int32
        ).reshape(-1, 3)
        npts = feats.shape[0]
        batch_ix = jnp.repeat(jnp.arange(b, dtype=jnp.int32), npts // b)
        nx = jnp.array([NXX, NXY, NZ], jnp.int32)
        kept = jnp.all((coords >= 0) & (coords < nx), axis=-1)
        lin = ((batch_ix * NZ + coords[:, 2]) * NXX + coords[:, 0]) * NXY + coords[:, 1]
        nseg = b * NZ * NXX * NXY
        lin = jnp.where(kept, lin, nseg)
        pooled = jax.ops.segment_sum(feats, lin, num_segments=nseg + 1)[:-1]
        out = pooled.reshape(b, NZ, NXX, NXY, c).transpose(0, 1, 4, 2, 3)
        final = out.reshape(b, NZ * c, NXX, NXY)
        return np.asarray(jax.device_get(final))


def plan(coords):
    """Build per-core mask/index tables from int voxel coords (vectorized).

    Returns None if the (d,w)/(d,h) factorization doesn't hold (caller then
    uses the host fallback), else a dict of planning tensors.
    """
    cx, cy, cz = coords[..., 0], coords[..., 1], coords[..., 2]
    if not (
        (cx == cx[:, :, :1, :]).all()
        and (cy == cy[:, :, :1, :]).all()
        and (cz == cz[:, :, :, :1]).all()
    ):
        return None

    vx = cx[:, :, 0, :].astype(np.int64)  # (B, D, W)
    vy = cy[:, :, 0, :].astype(np.int64)
    zk = cz[:, :, :, 0] == 0  # (B, D, H) keep mask

    inr = (vx >= 0) & (vx < NXX) & (vy >= 0) & (vy < NXY)
    vox = np.where(inr, vx * NXY + vy, -1)  # (B, D, W)

    # split into the two per-core w-halves: (B, 2, D, WS)
    v = vox.reshape(B, D, 2, WS).transpose(0, 2, 1, 3)

    # scan continuation mask: m=1 iff slot continues the same in-range voxel
    m = np.zeros((B, 2, D, WS), np.float32)
    m[..., 1:] = ((v[..., 1:] == v[..., :-1]) & (v[..., 1:] >= 0)).astype(
        np.float32
    )
    # run-end marker (where the scan state holds the full run total)
    lastw = np.ones((B, 2, D, WS), bool)
    lastw[..., :-1] = v[..., 1:] != v[..., :-1]

    # which 32400-row grid windows are touched by any core
    wins = tuple(sorted(np.unique(v[v >= 0] // WINR).tolist()))
    if not wins:
        wins = (0,)
    if len(wins) > 4:  # one SWDGE queue per window; ucode caps at 4
        return None

    # HW scatter-add races on duplicate indices within a call: a voxel must
    # not receive run totals from two different d-rows of the same core
    for b in range(B):
        for hf in range(2):
            vv = v[b, hf][lastw[b, hf] & (v[b, hf] >= 0)]
            if vv.size != np.unique(vv).size:
                return None

    # per-w scan mask (expanded to the (c w) layout on device)
    sm = m  # (B, 2, D, WS)

    # int16 scatter indices, one table per window, token i = w*128 + d.
    # Discarded tokens spread over the WINP-WINR spare rows: duplicate-index
    # RMW adds serialize per row on HW, so a single trash row would gate the
    # whole scatter.
    spread = TRASH + (np.arange(WS * 128, dtype=np.int16) % (WINP - WINR))
    idx = np.broadcast_to(
        spread.reshape(WS, 128), (B, 2, len(wins), WS, 128)
    ).copy()
    for j, k in enumerate(wins):
        real = lastw & (v >= 0) & (v // WINR == k)
        loc = np.where(real, v - k * WINR, 0).astype(np.int16)  # (B,2,D,WS)
        realT = real.transpose(0, 1, 3, 2)
        idx[:, :, j, :, :D] = np.where(
            realT, loc.transpose(0, 1, 3, 2), idx[:, :, j, :, :D]
        )
    # SBUF layout [16, ntok/16]: token i at partition i%16, column i//16;
    # the 16-partition table is then replicated 8x across 128 partitions
    # (one copy per gpsimd Q7 core, per the dma_scatter_add contract)
    idx = idx.reshape(B, 2, len(wins), NTOK // 16, 16).swapaxes(-1, -2)
    idx = idx.transpose(0, 1, 3, 2, 4).reshape(B, 2, 16, len(wins) * (NTOK // 16))
    idx = np.tile(idx, (1, 1, 8, 1))  # (B, 2, 128, nwin*NTOK/16)

    # PE h-mask, one 64-wide block per 4-d group. Group g accumulates into
    # PSUM rows [base, base+64) (base = 0 for g<16 else 64); within the block
    # only the group's own d-columns are nonzero:
    #   hm[b, g, 32*j + h, (4*g + j) - base] = zmask[4g+j, h]
    hm = np.zeros((B, GROUPS, 128, 64), np.float32)
    zkf = zk.astype(np.float32)
    for g in range(GROUPS):
        base = 0 if g < 16 else 64
        for j in range(min(4, D - 4 * g)):
            hm[:, g, 32 * j : 32 * j + H, 4 * g + j - base] = zkf[:, 4 * g + j, :]

    return {"wins": wins, "hm": hm, "sm": sm, "idx": idx}


def build_nc(nwin):
    """Build the (single, SPMD) Bass program for `nwin` scatter windows."""
    from concourse import bacc, bass, mybir
    from concourse import tile as tile_mod

    f32 = mybir.dt.float32
    bf16 = mybir.dt.bfloat16
    i16 = mybir.dt.int16

    nc = bacc.Bacc(
        trn_type="TRN2",
        target_bir_lowering=False,
        debug=False,
        enable_asserts=False,
        num_devices=N_CORES,
        dynamic_dma_scratch_size=1 << 15,
        num_swdge_queues=nwin,
    )
    # x packed as 15 super-groups of 8 d-slabs: [sg, p=(q h), s*WC + (c w)]
    # (d = 8*sg + 4*s + q; two 4-d halves per 1.76MB DMA tile)
    x_d = nc.dram_tensor("x_s", (SG, 128, 2 * WC), bf16, kind="ExternalInput")
    hm_d = nc.dram_tensor("hm", (128, GROUPS * 64), bf16, kind="ExternalInput")
    sm_d = nc.dram_tensor("sm", (D, WS), bf16, kind="ExternalInput")
    idx_d = nc.dram_tensor(
        "idx", (128, nwin * (NTOK // 16)), i16, kind="ExternalInput"
    )
    grid = nc.dram_tensor("grid", (nwin * WINP, 128), bf16, kind="ExternalOutput")

    sems = [nc.alloc_semaphore(f"scat_dma{q}") for q in range(nwin)]

    HC = WC // 2  # c-half split point in the (c w) layout

    with tile_mod.TileContext(nc) as tc:
        with (
            tc.tile_pool(name="const", bufs=1) as cp,
            tc.tile_pool(name="xp", bufs=4) as xp,
            tc.tile_pool(name="yp", bufs=1) as yp,
            tc.tile_pool(name="ps", bufs=1, space="PSUM") as pp,
        ):
            # small tables ride the sync queue ahead of the x stream: the
            # scalar queue's HWDGE stripes over only 2 DMA engines, and a
            # skewed engine finishes the stream ~10us after the rest
            hm_t = cp.tile([128, GROUPS * 64], bf16)
            nc.sync.dma_start(out=hm_t[:], in_=hm_d.ap())
            sm_s = cp.tile([128, WS], bf16)
            nc.sync.dma_start(out=sm_s[:D, :], in_=sm_d.ap())
            idx_t = cp.tile([128, nwin * (NTOK // 16)], i16)
            nc.sync.dma_start(out=idx_t[:], in_=idx_d.ap())
            sm_t = cp.tile([128, WC], bf16)

            y_ps = pp.tile([128, WC], f32)  # 7 PSUM banks, (c w) layout
            y_sa = yp.tile([128, HC], bf16, tag="ysa")  # scan out, c-half 0
            y_sb = yp.tile([128, WC - HC], bf16, tag="ysb")  # c-half 1
            y_t = yp.tile([128, WC], bf16, tag="yt")  # transposed, (w c)
            # pad partitions feed trash-row tokens; zero them once (engine
            # start-partition must be 32-aligned; rows 96..117 are rewritten
            # by the transpose copy afterwards)
            nc.gpsimd.memset(y_t[96:128, :], 0.0)

            def scan_half(out_t, c0, c1):
                # segmented run-sum along w; chains reset at w=0 of every c
                # (mask is 0 there), so c-column blocks split freely.
                # Separate output tiles per c-half let Tile see that the Act
                # transpose of half 0 only depends on the first scan.
                nc.vector.tensor_tensor_scan(
                    out=out_t[:D, :],
                    data0=sm_t[:D, c0:c1],
                    data1=y_ps[:D, c0:c1],
                    initial=0.0,
                    op0=mybir.AluOpType.mult,
                    op1=mybir.AluOpType.add,
                )

            def transpose_half(in_t, c0, c1):
                # (c w) -> (w c) on the Act engine, pipelined under the
                # DVE scan of the other c-half
                cc0, cc1 = c0 // WS, c1 // WS
                nc.scalar.copy(
                    out=y_t[:D].rearrange("p (w c) -> p w c", c=C)[
                        :, :, cc0:cc1
                    ],
                    in_=in_t[:D].rearrange("p (c w) -> p w c", w=WS),
                )

            xt = None
            for g in range(GROUPS):
                sg, half = g // 2, g % 2
                nd = min(4, D - 4 * g)
                rows = 32 * nd
                base = 0 if g < 16 else 64
                m = 64 if g < 16 else D - 64
                first = g in (0, 16)
                last = g in (15, GROUPS - 1)
                if half == 0:
                    xt = xp.tile([128, 2 * WC], bf16, tag="xt")
                    # alternate HWDGE queues so per-DMA setup overlaps the
                    # previous transfer
                    eng = nc.sync if sg % 2 == 0 else nc.scalar
                    eng.dma_start(out=xt[:], in_=x_d.ap()[sg])
                for n0 in range(0, WC, 512):
                    nn = min(512, WC - n0)
                    nc.tensor.matmul(
                        out=y_ps[base : base + m, n0 : n0 + nn],
                        lhsT=hm_t[:rows, g * 64 : g * 64 + m],
                        rhs=xt[:rows, half * WC + n0 : half * WC + n0 + nn],
                        start=first,
                        stop=last,
                    )
            # expand the per-w mask to the (c w) layout on-device (ships 80x
            # less table data). Emitted AFTER the matmuls: any earlier and
            # Tile's clock alignment makes the early matmuls wait on the DVE
            # queue (~16-21us PE stall at stream start). Its sm_s data dep
            # still lets it run early, long before the scan needs it.
            nc.vector.tensor_copy(
                out=sm_t[:D].rearrange("p (c w) -> p c w", c=C),
                in_=sm_s[:D, None, :].to_broadcast([D, C, WS]),
            )
            # dedup scan in two c-halves; each Act transpose runs under
            # the DVE scan of the other half
            scan_half(y_sa, 0, HC)
            scan_half(y_sb, HC, WC)
            transpose_half(y_sa, 0, HC)
            transpose_half(y_sb, HC, WC)

            # preps emitted HERE (not earlier): Tile's cross-engine clock
            # alignment otherwise makes the x-tile DMAs wait on the Pool
            # queue, which stalls ~40us behind the first prep's ucode init
            for j in range(nwin):
                nc.gpsimd.dma_scatter_add(
                    grid.ap()[j * WINP : (j + 1) * WINP, :C],
                    y_t[:].rearrange("p (w c) -> p w c", c=C),
                    idx_t[:, j * (NTOK // 16) : (j + 1) * (NTOK // 16)],
                    NTOK,
                    NTOK,
                    C,
                    elem_step=128,
                    prepare_only=True,
                    sem=sems[j],
                    queue_num=j,
                )
            for q in range(nwin):
                nc.gpsimd.trigger_dma(count=None, queue_num=q)

    nc.compile()
    return nc


def make_in_maps(x, p):
    """Per-core input dicts. Core i: batch i//2, w-half i%2."""
    import ml_dtypes

    bf16 = ml_dtypes.bfloat16
    x = np.asarray(x)
    in_maps = []
    for core in range(N_CORES):
        b, half = core // 2, core % 2
        # pack [d, h, w, c] -> [sg, (q h), (s c w)] with d = 8 sg + 4 s + q
        blk = x[b, :, :, half * WS : (half + 1) * WS, :]  # (D, H, WS, C)
        blk = np.concatenate(
            [blk, np.zeros((8 * SG - D,) + blk.shape[1:], blk.dtype)], axis=0
        )
        xs = (
            blk.transpose(0, 1, 3, 2)  # (D8, H, C, WS)
            .reshape(SG, 2, 4, H, C, WS)
            .transpose(0, 2, 3, 1, 4, 5)  # (sg, q, h, s, c, w)
            .reshape(SG, 128, 2 * WC)
            .astype(bf16, order="C")
        )
        in_maps.append(
            {
                "x_s": xs,
                "hm": p["hm"][b]
                .transpose(1, 0, 2)
                .reshape(128, GROUPS * 64)
                .astype(bf16, order="C"),
                "sm": p["sm"][b, half].astype(bf16, order="C"),
                "idx": np.ascontiguousarray(p["idx"][b, half]),
            }
        )
    return in_maps


def assemble(grids, wins):
    """grids: 8 (nwin*WINP, 128) bf16 arrays; w-half pairs add."""
    out = np.empty((B, C, NXX, NXY), np.float32)
    full = np.zeros((2, V, C), np.float32)
    for b in range(B):
        for half in range(2):
            g = np.asarray(grids[2 * b + half]).astype(np.float32)
            g = g.reshape(len(wins), WINP, 128)
            for j, k in enumerate(wins):
                full[half, k * WINR : (k + 1) * WINR] = g[j, :WINR, :C]
        s = full[0] + full[1]
        out[b] = s.reshape(NXX, NXY, C).transpose(2, 0, 1)
    return out


def _install_ntff_shim():
    """Provide antenv.axon_hooks with an NTFF profile hook driven by ctypes
    into the axon PJRT .so (the agent image's antenv lacks axon_hooks; this
    replicates trn_agent_boot's degraded-away hook). Only used when
    KERNEL_TRACE=1."""
    import contextlib
    import ctypes
    import types

    if "antenv.axon_hooks" in sys.modules:
        return
    so_path = "/opt/axon/libaxon_pjrt.so"
    if not os.path.exists(so_path):
        return
    lib = ctypes.CDLL(so_path)
    if not hasattr(lib, "axon_start_nrt_profile"):
        return
    lib.axon_start_nrt_profile.argtypes = [
        ctypes.POINTER(ctypes.c_int64),
        ctypes.c_size_t,
    ]
    lib.axon_start_nrt_profile.restype = ctypes.c_int64
    lib.axon_stop_nrt_profile.argtypes = [ctypes.c_char_p]
    lib.axon_stop_nrt_profile.restype = ctypes.c_int64

    @contextlib.contextmanager
    def _hook(output_dir, device_ids):
        import jax

        jax.devices()
        if device_ids:
            ids = (ctypes.c_int64 * len(device_ids))(*device_ids)
            rc = lib.axon_start_nrt_profile(ids, len(device_ids))
        else:
            rc = lib.axon_start_nrt_profile(None, 0)
        if rc != 0:
            raise RuntimeError(f"axon_start_nrt_profile rc={rc}")
        try:
            yield
        finally:
            n = lib.axon_stop_nrt_profile(str(output_dir).encode())
            print(f"ntff profile: {n} file(s) written to {output_dir}")

    mod = types.ModuleType("antenv.axon_hooks")
    mod.get_axon_ntff_profile_hook = lambda: _hook
    mod.set_axon_ntff_profile_hook = lambda h: None
    sys.modules["antenv.axon_hooks"] = mod


def kernel(**inputs):
    x = np.asarray(inputs["x"])
    coords = _host_coords(**inputs)
    p = plan(coords)
    if p is None:
        return _host_fallback(**inputs)

    wins = p["wins"]
    if wins not in _NC_CACHE:
        _NC_CACHE[wins] = build_nc(len(wins))
    nc = _NC_CACHE[wins]

    from concourse.bass_utils import run_bass_kernel_spmd

    trace = bool(int(os.environ.get("KERNEL_TRACE", "0")))
    trace_cores = None
    if trace:
        tc_env = os.environ.get("KERNEL_TRACE_CORES", "0")
        trace_cores = [int(t) for t in tc_env.split(",") if t != ""]
        _install_ntff_shim()
    res = run_bass_kernel_spmd(
        nc,
        make_in_maps(x, p),
        core_ids=list(range(N_CORES)),
        trace=trace,
        trace_cores=trace_cores,
    )
    kernel.last_results = res
    if res.exec_time_ns is not None:
        print(f"HW exec time: {res.exec_time_ns} ns")
    grids = [res.results[i]["grid"] for i in range(N_CORES)]
    return assemble(grids, wins)


kernel.last_results = None



# revision 3
# speedup vs baseline: 1.1278x; 1.1278x over previous
"""BEV pooling (LSS view transform) kernel for Trainium2, 8 NeuronCores.

Problem: x (B=4, D=118, H=32, W=88, C=80) camera frustum features are pooled
into a (B, C, 360, 360) BEV grid via voxel scatter-add (segment_sum).

Structure exploited (verified at runtime from the actual inputs):
  - camera->lidar maps pixel (u, v, d): BEV voxel depends on (d, w) only and
    the z-range keep-mask on (d, h) only.
  - So  pooled[vox(d,w)] += sum_h zmask(d,h) * x[d,h,w,:], and within a d-row
    equal-voxel groups are consecutive runs in w.

Device kernel per core (core = one batch x one 44-column w-half; runs that
cross the w boundary give partial sums in each core's private grid, which the
host adds):
  Stage A: stream x in fp8-e3m4 [128, 7040] tiles (near-field supergroups
           that feed the hottest voxels stay bf16 for precision), laid out
           (d h)(c w); PE matmul with a block-diagonal 0/1 h-mask reduces
           over h into one PSUM tile y[118, (c w)] (d<64 at quadrant 0,
           d>=64 at quadrant 64 -- the quadrant halves finalize at ~56% /
           100% of the stream respectively).
  Stage B: per quadrant, a DVE tensor_tensor_scan (state = m*state + y,
           fp32 state) along the w-innermost free dim computes every run's
           total at its run-END slot; strided tensor_copies transpose
           (c w) -> (w c).  The d<64 quadrant's scan+transpose runs while
           the d>=64 matmuls still stream.
  Stage C: dma_scatter_add preps in prepare_only mode on FOUR SWDGE queues
           (one per 11-wide w-range).  The grid uses a per-core LOCAL row
           space (distinct run-end voxels, ~4k rows << int16 range), so one
           8192-row window covers everything; the host assemble maps local
           rows back to global voxels.  Non-run-end / out-of-range tokens
           aim at spread-out trash rows above the real rows.

The grid DRAM tensor is pre-zeroed by the runner (documented contract of
run_bass_kernel_spmd), so untouched voxels read 0 and a single scatter-ADD
per real voxel equals a plain write.
"""

import os
import sys

import numpy as np

sys.path.insert(0, "/opt/trn_rl_repo")

# ---- problem constants (hardcoded per spec) ----
B, D, H, W, C = 4, 118, 32, 88, 80
WS = W // 2  # per-core w-column span (cores shard on batch x w-half)
NXX = NXY = 360
NZ = 1
V = NXX * NXY  # voxels per batch slice
DX = np.array([0.3, 0.3, 20.0], np.float32)
BX_LO = np.array([-54.0, -54.0, -10.0], np.float32)
N_CORES = 8
GROUPS = (D + 3) // 4  # 30 groups of <=4 d-slabs
WC = WS * C  # 3520

LSIZE = 8192  # local grid rows per core (distinct vox + trash spread)
NQ = 4  # SWDGE scatter queues (w-ranges of 11 columns each)
WQ = WS // NQ  # w columns per queue
NTOKQ = WQ * 128  # scatter tokens per queue call
SG = 15  # 8-d super-groups per core (d padded 118 -> 120)

_NC_CACHE: dict = {}


def _host_coords(x, camera2lidar_rots, camera2lidar_trans, intrins, frustum):
    """Voxel int coords for every point, bit-identical to the reference
    (same jax ops on the cpu backend)."""
    import jax
    import jax.numpy as jnp

    cpu = jax.devices("cpu")[0]
    with jax.default_device(cpu):
        frustum = jnp.asarray(np.asarray(frustum))
        rots = jnp.asarray(np.asarray(camera2lidar_rots))
        trans = jnp.asarray(np.asarray(camera2lidar_trans))
        intr = jnp.asarray(np.asarray(intrins))
        pts = jnp.concatenate(
            [frustum[..., :2] * frustum[..., 2:3], frustum[..., 2:3]], axis=-1
        )
        combine = rots @ jnp.linalg.inv(intr)
        geom = (
            jnp.einsum("bij,dhwj->bdhwi", combine, pts)
            + trans[:, None, None, None, :]
        )
        coords = ((geom - jnp.asarray(BX_LO)) / jnp.asarray(DX)).astype(jnp.int32)
        coords = np.asarray(jax.device_get(coords))
    return coords  # (B, D, H, W, 3) int32


def _host_fallback(x, camera2lidar_rots, camera2lidar_trans, intrins, frustum):
    """Exact reference computation on host (jax cpu). Correct for arbitrary
    inputs; used only if the factorized structure doesn't hold."""
    import jax
    import jax.numpy as jnp

    cpu = jax.devices("cpu")[0]
    with jax.default_device(cpu):
        x = jnp.asarray(np.asarray(x))
        rots = jnp.asarray(np.asarray(camera2lidar_rots))
        trans = jnp.asarray(np.asarray(camera2lidar_trans))
        intr = jnp.asarray(np.asarray(intrins))
        frustum = jnp.asarray(np.asarray(frustum))
        b, d, h, w, c = x.shape
        pts = jnp.concatenate(
            [frustum[..., :2] * frustum[..., 2:3], frustum[..., 2:3]], axis=-1
        )
        combine = rots @ jnp.linalg.inv(intr)
        geom = (
            jnp.einsum("bij,dhwj->bdhwi", combine, pts)
            + trans[:, None, None, None, :]
        )
        feats = x.reshape(-1, c)
        coords = ((geom - jnp.asarray(BX_LO)) / jnp.asarray(DX)).astype(
            jnp.int32
        ).reshape(-1, 3)
        npts = feats.shape[0]
        batch_ix = jnp.repeat(jnp.arange(b, dtype=jnp.int32), npts // b)
        nx = jnp.array([NXX, NXY, NZ], jnp.int32)
        kept = jnp.all((coords >= 0) & (coords < nx), axis=-1)
        lin = ((batch_ix * NZ + coords[:, 2]) * NXX + coords[:, 0]) * NXY + coords[:, 1]
        nseg = b * NZ * NXX * NXY
        lin = jnp.where(kept, lin, nseg)
        pooled = jax.ops.segment_sum(feats, lin, num_segments=nseg + 1)[:-1]
        out = pooled.reshape(b, NZ, NXX, NXY, c).transpose(0, 1, 4, 2, 3)
        final = out.reshape(b, NZ * c, NXX, NXY)
        return np.asarray(jax.device_get(final))


def plan(coords, x):
    """Build per-core mask/index tables from int voxel coords (vectorized).

    Returns None if the (d,w)/(d,h) factorization doesn't hold (caller then
    uses the host fallback), else a dict of planning tensors.
    """
    cx, cy, cz = coords[..., 0], coords[..., 1], coords[..., 2]
    if not (
        (cx == cx[:, :, :1, :]).all()
        and (cy == cy[:, :, :1, :]).all()
        and (cz == cz[:, :, :, :1]).all()
    ):
        return None

    vx = cx[:, :, 0, :].astype(np.int64)  # (B, D, W)
    vy = cy[:, :, 0, :].astype(np.int64)
    zk = cz[:, :, :, 0] == 0  # (B, D, H) keep mask

    inr = (vx >= 0) & (vx < NXX) & (vy >= 0) & (vy < NXY)
    vox = np.where(inr, vx * NXY + vy, -1)  # (B, D, W)

    # split into the two per-core w-halves: (B, 2, D, WS)
    v = vox.reshape(B, D, 2, WS).transpose(0, 2, 1, 3)

    # scan continuation mask: m=1 iff slot continues the same in-range voxel
    m = np.zeros((B, 2, D, WS), np.float32)
    m[..., 1:] = ((v[..., 1:] == v[..., :-1]) & (v[..., 1:] >= 0)).astype(
        np.float32
    )
    # run-end marker (where the scan state holds the full run total)
    lastw = np.ones((B, 2, D, WS), bool)
    lastw[..., :-1] = v[..., 1:] != v[..., :-1]

    # HW scatter-add races on duplicate indices within a concurrent queue
    # set: a voxel must not receive run totals from two different d-rows of
    # the same core.  (This also guarantees local rows are disjoint across
    # the 4 w-range queues.)  Per-core local row maps follow.
    uvoxes = []
    for b in range(B):
        for hf in range(2):
            real = lastw[b, hf] & (v[b, hf] >= 0)
            vv = v[b, hf][real]
            uv = np.unique(vv)
            if vv.size != uv.size:
                return None
            if uv.size > LSIZE - 1024:  # need trash-row headroom
                return None
            uvoxes.append(uv)

    # int16 scatter indices in LOCAL row space, one table slab per queue,
    # token i = w_local*128 + d within the queue's 11-wide w-range.
    # Discarded tokens spread over the spare rows above the real rows:
    # duplicate-index RMW adds serialize per row on HW, so a single trash
    # row would gate the whole scatter.
    idx = np.zeros((B, 2, NQ, WQ, 128), np.int16)
    for b in range(B):
        for hf in range(2):
            uv = uvoxes[2 * b + hf]
            nv = uv.size
            spread = nv + (
                np.arange(NQ * WQ * 128, dtype=np.int64) % (LSIZE - nv)
            )
            tab = spread.reshape(NQ, WQ, 128).astype(np.int16)
            real = lastw[b, hf] & (v[b, hf] >= 0)  # (D, WS)
            loc = np.zeros((D, WS), np.int64)
            loc[real] = np.searchsorted(uv, v[b, hf][real])
            realT = real.T.reshape(NQ, WQ, D)  # (NQ, WQ, D)
            locT = loc.T.reshape(NQ, WQ, D)
            tab[:, :, :D] = np.where(realT, locT, tab[:, :, :D]).astype(np.int16)
            idx[b, hf] = tab

    # SBUF layout [16, ntok/16]: token i at partition i%16, column i//16;
    # the 16-partition table is then replicated 8x across 128 partitions
    # (one copy per gpsimd Q7 core, per the dma_scatter_add contract)
    idx = idx.reshape(B, 2, NQ, NTOKQ // 16, 16).swapaxes(-1, -2)
    idx = idx.transpose(0, 1, 3, 2, 4).reshape(B, 2, 16, NQ * (NTOKQ // 16))
    idx = np.tile(idx, (1, 1, 8, 1))  # (B, 2, 128, NQ*NTOKQ/16)

    # per-w scan mask (expanded to the (c w) layout on device)
    sm = m  # (B, 2, D, WS)

    # PE h-mask, one 64-wide block per 4-d group. Group g accumulates into
    # PSUM rows [base, base+64) (base = 0 for g<16 else 64); within the block
    # only the group's own d-columns are nonzero:
    #   hm[b, g, 32*j + h, (4*g + j) - base] = zmask[4g+j, h]
    hm = np.zeros((B, GROUPS, 128, 64), np.float32)
    zkf = zk.astype(np.float32)
    for g in range(GROUPS):
        base = 0 if g < 16 else 64
        for j in range(min(4, D - 4 * g)):
            hm[:, g, 32 * j : 32 * j + H, 4 * g + j - base] = zkf[:, 4 * g + j, :]

    # fp8 hybrid: supergroups feeding voxels with many contributions keep
    # bf16 (quantization noise accumulates as sqrt(n) over n summed points).
    # Contribution counts per voxel, then per-supergroup contribution to the
    # hot set; promote supergroups until the predicted noise is comfortable.
    wt = zk.sum(-1)  # (B, D) kept-h count
    promo = set()
    for b in range(B):
        acc = np.zeros(V, np.int64)
        w_pts = np.where(vox[b] >= 0, 1, 0) * wt[b][:, None]
        np.add.at(
            acc,
            np.where(vox[b] >= 0, vox[b], 0),
            np.where(vox[b] >= 0, w_pts, 0),
        )
        hotset = acc > 300
        if not hotset.any():
            continue
        mask_hot = np.where(vox[b] >= 0, hotset[np.where(vox[b] >= 0, vox[b], 0)], False)
        contrib = (mask_hot * w_pts).sum(-1)  # (D,)
        sg_contrib = np.zeros(SG)
        for sg in range(SG):
            sg_contrib[sg] = contrib[8 * sg : min(8 * sg + 8, D)].sum()
        tot = sg_contrib.sum()
        if tot == 0:
            continue
        order = np.argsort(-sg_contrib)
        run = 0.0
        for sg in order:
            if sg_contrib[sg] == 0 or run >= 0.95 * tot:
                break
            promo.add(int(sg))
            run += sg_contrib[sg]
    promo = tuple(sorted(promo))

    return {"hm": hm, "sm": sm, "idx": idx, "uvoxes": uvoxes, "promo": promo}


def build_nc(promo):
    """Build the (single, SPMD) Bass program; `promo` = bf16 supergroups."""
    from concourse import bacc, bass, mybir
    from concourse import tile as tile_mod

    f32 = mybir.dt.float32
    bf16 = mybir.dt.bfloat16
    fp8 = mybir.dt.float8e3
    i16 = mybir.dt.int16

    promo = set(promo)
    n16 = len(promo)
    n8 = SG - n16

    nc = bacc.Bacc(
        trn_type="TRN2",
        target_bir_lowering=False,
        debug=False,
        enable_asserts=False,
        num_devices=N_CORES,
        dynamic_dma_scratch_size=1 << 15,
        num_swdge_queues=NQ,
    )
    # x packed as supergroups of 8 d-slabs: [sg, p=(q h), s*WC + (c w)]
    # (d = 8*sg + 4*s + q; two 4-d halves per DMA tile).  bf16 supergroups
    # and fp8 supergroups live in separate dram tensors.
    x8_d = x16_d = None
    if n8:
        x8_d = nc.dram_tensor("x8", (n8, 128, 2 * WC), fp8, kind="ExternalInput")
    if n16:
        x16_d = nc.dram_tensor(
            "x16", (n16, 128, 2 * WC), bf16, kind="ExternalInput"
        )
    hm8_d = hm16_d = None
    if n8:
        hm8_d = nc.dram_tensor("hm8", (128, 2 * n8 * 64), fp8, kind="ExternalInput")
    if n16:
        hm16_d = nc.dram_tensor(
            "hm16", (128, 2 * n16 * 64), bf16, kind="ExternalInput"
        )
    sm_d = nc.dram_tensor("sm", (D, WS), bf16, kind="ExternalInput")
    idx_d = nc.dram_tensor(
        "idx", (128, NQ * (NTOKQ // 16)), i16, kind="ExternalInput"
    )
    grid = nc.dram_tensor("grid", (LSIZE, 128), bf16, kind="ExternalOutput")

    sems = [nc.alloc_semaphore(f"scat_dma{q}") for q in range(NQ)]

    HC = WC // 2  # c-half split point in the (c w) layout

    with tile_mod.TileContext(nc) as tc:
        with (
            tc.tile_pool(name="const", bufs=1) as cp,
            tc.tile_pool(name="xp", bufs=4) as xp,
            tc.tile_pool(name="yp", bufs=1) as yp,
            tc.tile_pool(name="ps", bufs=1, space="PSUM") as pp,
        ):
            # small tables ride the sync queue ahead of the x stream
            hm8_t = hm16_t = None
            if n8:
                hm8_t = cp.tile([128, 2 * n8 * 64], fp8)
                nc.sync.dma_start(out=hm8_t[:], in_=hm8_d.ap())
            if n16:
                hm16_t = cp.tile([128, 2 * n16 * 64], bf16)
                nc.sync.dma_start(out=hm16_t[:], in_=hm16_d.ap())
            sm_s = cp.tile([128, WS], bf16)
            nc.sync.dma_start(out=sm_s[:D, :], in_=sm_d.ap())
            idx_t = cp.tile([128, NQ * (NTOKQ // 16)], i16)
            nc.sync.dma_start(out=idx_t[:], in_=idx_d.ap())
            sm_t = cp.tile([128, WC], bf16)

            y_ps = pp.tile([128, WC], f32)  # 7 PSUM banks, (c w) layout
            # scan outputs, one tile per (quadrant, c-half) so the Act/Pool
            # transposes pair with exactly the scan that produced their input
            y_sa0 = yp.tile([128, HC], bf16, tag="ysa0")  # rows 0:64, c-half 0
            y_sa1 = yp.tile([128, WC - HC], bf16, tag="ysa1")
            y_sb0 = yp.tile([128, HC], bf16, tag="ysb0")  # rows 64:118
            y_sb1 = yp.tile([128, WC - HC], bf16, tag="ysb1")
            y_t = yp.tile([128, WC], bf16, tag="yt")  # transposed, (w c)
            # pad partitions feed trash-row tokens; zero them once (engine
            # start-partition must be 32-aligned; rows 96..117 are rewritten
            # by the transpose copy afterwards)
            nc.gpsimd.memset(y_t[96:128, :], 0.0)

            # counts of fp8/bf16 supergroups seen so far, to index the right
            # dram tensor; stream order stays d-ascending regardless
            i8 = 0
            i16_ = 0
            xt = None
            x_is16 = False
            for g in range(GROUPS):
                sg, half = g // 2, g % 2
                nd = min(4, D - 4 * g)
                rows = 32 * nd
                base = 0 if g < 16 else 64
                mrows = 64 if g < 16 else D - 64
                first = g in (0, 16)
                last = g in (15, GROUPS - 1)
                if half == 0:
                    x_is16 = sg in promo
                    dt = bf16 if x_is16 else fp8
                    xt = xp.tile([128, 2 * WC], dt, tag="xt")
                    # alternate HWDGE queues so per-DMA setup overlaps the
                    # previous transfer
                    eng = nc.sync if sg % 2 == 0 else nc.scalar
                    if x_is16:
                        eng.dma_start(out=xt[:], in_=x16_d.ap()[i16_])
                        i16_ += 1
                    else:
                        eng.dma_start(out=xt[:], in_=x8_d.ap()[i8])
                        i8 += 1
                hm_t = hm16_t if x_is16 else hm8_t
                hcol = ((i16_ - 1) if x_is16 else (i8 - 1)) * 128 + half * 64
                for n0 in range(0, WC, 512):
                    nn = min(512, WC - n0)
                    nc.tensor.matmul(
                        out=y_ps[base : base + mrows, n0 : n0 + nn],
                        lhsT=hm_t[:rows, hcol : hcol + mrows],
                        rhs=xt[:rows, half * WC + n0 : half * WC + n0 + nn],
                        start=first,
                        stop=last,
                    )
            # expand the per-w mask to the (c w) layout on-device (ships 80x
            # less table data). Emitted AFTER the matmuls: any earlier and
            # Tile's clock alignment makes the early matmuls wait on the DVE
            # queue (~16-21us PE stall at stream start). Its sm_s data dep
            # still lets it run early, long before the scan needs it.
            nc.vector.tensor_copy(
                out=sm_t[:D].rearrange("p (c w) -> p c w", c=C),
                in_=sm_s[:D, None, :].to_broadcast([D, C, WS]),
            )

            def scan_part(out_t, r0, r1, c0, c1):
                # segmented run-sum along w; chains reset at w=0 of every c
                # (mask is 0 there), so c-column blocks split freely.
                nc.vector.tensor_tensor_scan(
                    out=out_t[r0:r1, : c1 - c0],
                    data0=sm_t[r0:r1, c0:c1],
                    data1=y_ps[r0:r1, c0:c1],
                    initial=0.0,
                    op0=mybir.AluOpType.mult,
                    op1=mybir.AluOpType.add,
                )

            def transpose_part(eng, in_t, r0, r1, c0, c1):
                # (c w) -> (w c) strided copy, pipelined under later scans
                cc0, cc1 = c0 // WS, c1 // WS
                out_ap = y_t[r0:r1].rearrange("p (w c) -> p w c", c=C)[
                    :, :, cc0:cc1
                ]
                in_ap = in_t[r0:r1, : c1 - c0].rearrange("p (c w) -> p w c", w=WS)
                if eng is nc.scalar:
                    eng.copy(out=out_ap, in_=in_ap)
                else:
                    eng.tensor_copy(out=out_ap, in_=in_ap)

            # quadrant A (rows 0:64) finalizes in PSUM when group 15 stops,
            # ~56% through the stream; its scans+transposes overlap the
            # quadrant-B matmuls.  Scans split per c-half so each transpose
            # runs under the next scan; transposes split Act/Pool to halve
            # the exposed tail.
            scan_part(y_sa0, 0, 64, 0, HC)
            scan_part(y_sa1, 0, 64, HC, WC)
            transpose_part(nc.scalar, y_sa0, 0, 64, 0, HC)
            transpose_part(nc.gpsimd, y_sa1, 0, 64, HC, WC)
            scan_part(y_sb0, 64, D, 0, HC)
            scan_part(y_sb1, 64, D, HC, WC)
            transpose_part(nc.scalar, y_sb0, 64, D, 0, HC)
            transpose_part(nc.gpsimd, y_sb1, 64, D, HC, WC)

            # preps emitted HERE (not earlier): Tile's cross-engine clock
            # alignment otherwise makes the x-tile DMAs wait on the Pool
            # queue, which stalls ~40us behind the first prep's ucode init.
            # Each queue covers an 11-wide w-range; local rows are disjoint
            # across queues (run-end voxels are unique core-wide), so the
            # four RMW streams cannot race.
            for q in range(NQ):
                nc.gpsimd.dma_scatter_add(
                    grid.ap()[:, :C],
                    y_t[:, q * WQ * C : (q + 1) * WQ * C].rearrange(
                        "p (w c) -> p w c", c=C
                    ),
                    idx_t[:, q * (NTOKQ // 16) : (q + 1) * (NTOKQ // 16)],
                    NTOKQ,
                    NTOKQ,
                    C,
                    elem_step=128,
                    prepare_only=True,
                    sem=sems[q],
                    queue_num=q,
                )
            for q in range(NQ):
                nc.gpsimd.trigger_dma(count=None, queue_num=q)

    nc.compile()
    return nc


def make_in_maps(x, p):
    """Per-core input dicts. Core i: batch i//2, w-half i%2."""
    import ml_dtypes

    bf16 = ml_dtypes.bfloat16
    fp8 = ml_dtypes.float8_e3m4
    promo = set(p["promo"])
    x = np.asarray(x)
    in_maps = []
    for core in range(N_CORES):
        b, half = core // 2, core % 2
        # pack [d, h, w, c] -> [sg, (q h), (s c w)] with d = 8 sg + 4 s + q
        blk = x[b, :, :, half * WS : (half + 1) * WS, :]  # (D, H, WS, C)
        blk = np.concatenate(
            [blk, np.zeros((8 * SG - D,) + blk.shape[1:], blk.dtype)], axis=0
        )
        xs = (
            blk.transpose(0, 1, 3, 2)  # (D8, H, C, WS)
            .reshape(SG, 2, 4, H, C, WS)
            .transpose(0, 2, 3, 1, 4, 5)  # (sg, q, h, s, c, w)
            .reshape(SG, 128, 2 * WC)
        )
        sg8 = [s for s in range(SG) if s not in promo]
        sg16 = [s for s in range(SG) if s in promo]
        # hm packed per dtype in stream order: [128, nsg*2*64] with the two
        # 4-d group halves of each supergroup adjacent
        hmb = p["hm"][b]  # (GROUPS, 128, 64) fp32
        hm_sg = np.concatenate(
            [hmb[2 * s : 2 * s + 2].transpose(1, 0, 2).reshape(128, 128) for s in range(SG)],
            axis=1,
        ).reshape(128, SG, 128)
        im = {
            "sm": p["sm"][b, half].astype(bf16, order="C"),
            "idx": np.ascontiguousarray(p["idx"][b, half]),
        }
        if sg8:
            im["x8"] = np.ascontiguousarray(xs[sg8]).astype(fp8, order="C")
            im["hm8"] = np.ascontiguousarray(
                hm_sg[:, sg8].reshape(128, -1)
            ).astype(fp8, order="C")
        if sg16:
            im["x16"] = np.ascontiguousarray(xs[sg16]).astype(bf16, order="C")
            im["hm16"] = np.ascontiguousarray(
                hm_sg[:, sg16].reshape(128, -1)
            ).astype(bf16, order="C")
        in_maps.append(im)
    return in_maps


def assemble(grids, uvoxes):
    """grids: 8 (LSIZE, 128) bf16 arrays in per-core local row space;
    w-half pairs add into the shared (V, C) batch grid."""
    out = np.empty((B, C, NXX, NXY), np.float32)
    for b in range(B):
        s = np.zeros((V, C), np.float32)
        for half in range(2):
            uv = uvoxes[2 * b + half]
            g = np.asarray(grids[2 * b + half])[: uv.size, :C].astype(np.float32)
            s[uv] += g
        out[b] = s.reshape(NXX, NXY, C).transpose(2, 0, 1)
    return out


def _install_ntff_shim():
    """Provide antenv.axon_hooks with an NTFF profile hook driven by ctypes
    into the axon PJRT .so (the agent image's antenv lacks axon_hooks; this
    replicates trn_agent_boot's degraded-away hook). Only used when
    KERNEL_TRACE=1."""
    import contextlib
    import ctypes
    import types

    if "antenv.axon_hooks" in sys.modules:
        return
    so_path = "/opt/axon/libaxon_pjrt.so"
    if not os.path.exists(so_path):
        return
    lib = ctypes.CDLL(so_path)
    if not hasattr(lib, "axon_start_nrt_profile"):
        return
    lib.axon_start_nrt_profile.argtypes = [
        ctypes.POINTER(ctypes.c_int64),
        ctypes.c_size_t,
    ]
    lib.axon_start_nrt_profile.restype = ctypes.c_int64
    lib.axon_stop_nrt_profile.argtypes = [ctypes.c_char_p]
    lib.axon_stop_nrt_profile.restype = ctypes.c_int64

    @contextlib.contextmanager
    def _hook(output_dir, device_ids):
        import jax

        jax.devices()
        if device_ids:
            ids = (ctypes.c_int64 * len(device_ids))(*device_ids)
            rc = lib.axon_start_nrt_profile(ids, len(device_ids))
        else:
            rc = lib.axon_start_nrt_profile(None, 0)
        if rc != 0:
            raise RuntimeError(f"axon_start_nrt_profile rc={rc}")
        try:
            yield
        finally:
            n = lib.axon_stop_nrt_profile(str(output_dir).encode())
            print(f"ntff profile: {n} file(s) written to {output_dir}")

    mod = types.ModuleType("antenv.axon_hooks")
    mod.get_axon_ntff_profile_hook = lambda: _hook
    mod.set_axon_ntff_profile_hook = lambda h: None
    sys.modules["antenv.axon_hooks"] = mod


def kernel(**inputs):
    x = np.asarray(inputs["x"])
    coords = _host_coords(**inputs)
    p = plan(coords, x)
    if p is None:
        return _host_fallback(**inputs)

    key = p["promo"]
    if key not in _NC_CACHE:
        _NC_CACHE[key] = build_nc(key)
    nc = _NC_CACHE[key]

    from concourse.bass_utils import run_bass_kernel_spmd

    trace = bool(int(os.environ.get("KERNEL_TRACE", "0")))
    trace_cores = None
    if trace:
        tc_env = os.environ.get("KERNEL_TRACE_CORES", "0")
        trace_cores = [int(t) for t in tc_env.split(",") if t != ""]
        _install_ntff_shim()
    res = run_bass_kernel_spmd(
        nc,
        make_in_maps(x, p),
        core_ids=list(range(N_CORES)),
        trace=trace,
        trace_cores=trace_cores,
    )
    kernel.last_results = res
    if res.exec_time_ns is not None:
        print(f"HW exec time: {res.exec_time_ns} ns")
    grids = [res.results[i]["grid"] for i in range(N_CORES)]
    return assemble(grids, p["uvoxes"])


kernel.last_results = None


# revision 7
# speedup vs baseline: 1.3716x; 1.2162x over previous
"""BEV pooling (LSS view transform) kernel for Trainium2, 8 NeuronCores.

Problem: x (B=4, D=118, H=32, W=88, C=80) camera frustum features are pooled
into a (B, C, 360, 360) BEV grid via voxel scatter-add (segment_sum).

Structure exploited (verified at runtime from the actual inputs):
  - camera->lidar maps pixel (u, v, d): BEV voxel depends on (d, w) only and
    the z-range keep-mask on (d, h) only.
  - So  pooled[vox(d,w)] += sum_h zmask(d,h) * x[d,h,w,:], and within a d-row
    equal-voxel groups are consecutive runs in w.

Device kernel per core (core = one batch x one 44-column w-half; runs that
cross the w boundary give partial sums in each core's private grid, which the
host adds):
  Stage A: stream x in fp8-e3m4 [128, 7040] tiles (near-field supergroups
           that feed the hottest voxels stay bf16 for precision), laid out
           (d h)(c w); PE matmul with a block-diagonal 0/1 h-mask reduces
           over h into one PSUM tile y[118, (c w)] (d<64 at quadrant 0,
           d>=64 at quadrant 64 -- the quadrant halves finalize at ~56% /
           100% of the stream respectively).
  Stage B: per quadrant, a DVE tensor_tensor_scan (state = m*state + y,
           fp32 state) along the w-innermost free dim computes every run's
           total at its run-END slot; strided tensor_copies transpose
           (c w) -> (w c).  The d<64 quadrant's scan+transpose runs while
           the d>=64 matmuls still stream.
  Stage C: dma_scatter_add preps in prepare_only mode on FOUR SWDGE queues
           (one per 11-wide w-range).  The grid uses a per-core LOCAL row
           space (distinct run-end voxels, ~4k rows << int16 range), so one
           8192-row window covers everything; the host assemble maps local
           rows back to global voxels.  Non-run-end / out-of-range tokens
           aim at spread-out trash rows above the real rows.

The grid DRAM tensor is pre-zeroed by the runner (documented contract of
run_bass_kernel_spmd), so untouched voxels read 0 and a single scatter-ADD
per real voxel equals a plain write.
"""

import os
import sys

import numpy as np

sys.path.insert(0, "/opt/trn_rl_repo")

# ---- problem constants (hardcoded per spec) ----
B, D, H, W, C = 4, 118, 32, 88, 80
WS = W // 2  # per-core w-column span (cores shard on batch x w-half)
NXX = NXY = 360
NZ = 1
V = NXX * NXY  # voxels per batch slice
DX = np.array([0.3, 0.3, 20.0], np.float32)
BX_LO = np.array([-54.0, -54.0, -10.0], np.float32)
N_CORES = 8
GROUPS = (D + 3) // 4  # 30 groups of <=4 d-slabs
WC = WS * C  # 3520

LSIZE = 8192  # local grid rows per core (distinct vox + trash spread)
NQ = 4  # SWDGE scatter queues (w-ranges of 11 columns each)
WQ = WS // NQ  # w columns per queue
NTOKQ = WQ * 128  # scatter tokens per queue call
SG = 15  # 8-d super-groups per core (d padded 118 -> 120)

_NC_CACHE: dict = {}


def _host_coords(x, camera2lidar_rots, camera2lidar_trans, intrins, frustum):
    """Voxel int coords for every point, bit-identical to the reference
    (same jax ops on the cpu backend)."""
    import jax
    import jax.numpy as jnp

    cpu = jax.devices("cpu")[0]
    with jax.default_device(cpu):
        frustum = jnp.asarray(np.asarray(frustum))
        rots = jnp.asarray(np.asarray(camera2lidar_rots))
        trans = jnp.asarray(np.asarray(camera2lidar_trans))
        intr = jnp.asarray(np.asarray(intrins))
        pts = jnp.concatenate(
            [frustum[..., :2] * frustum[..., 2:3], frustum[..., 2:3]], axis=-1
        )
        combine = rots @ jnp.linalg.inv(intr)
        geom = (
            jnp.einsum("bij,dhwj->bdhwi", combine, pts)
            + trans[:, None, None, None, :]
        )
        coords = ((geom - jnp.asarray(BX_LO)) / jnp.asarray(DX)).astype(jnp.int32)
        coords = np.asarray(jax.device_get(coords))
    return coords  # (B, D, H, W, 3) int32


def _host_fallback(x, camera2lidar_rots, camera2lidar_trans, intrins, frustum):
    """Exact reference computation on host (jax cpu). Correct for arbitrary
    inputs; used only if the factorized structure doesn't hold."""
    import jax
    import jax.numpy as jnp

    cpu = jax.devices("cpu")[0]
    with jax.default_device(cpu):
        x = jnp.asarray(np.asarray(x))
        rots = jnp.asarray(np.asarray(camera2lidar_rots))
        trans = jnp.asarray(np.asarray(camera2lidar_trans))
        intr = jnp.asarray(np.asarray(intrins))
        frustum = jnp.asarray(np.asarray(frustum))
        b, d, h, w, c = x.shape
        pts = jnp.concatenate(
            [frustum[..., :2] * frustum[..., 2:3], frustum[..., 2:3]], axis=-1
        )
        combine = rots @ jnp.linalg.inv(intr)
        geom = (
            jnp.einsum("bij,dhwj->bdhwi", combine, pts)
            + trans[:, None, None, None, :]
        )
        feats = x.reshape(-1, c)
        coords = ((geom - jnp.asarray(BX_LO)) / jnp.asarray(DX)).astype(
            jnp.int32
        ).reshape(-1, 3)
        npts = feats.shape[0]
        batch_ix = jnp.repeat(jnp.arange(b, dtype=jnp.int32), npts // b)
        nx = jnp.array([NXX, NXY, NZ], jnp.int32)
        kept = jnp.all((coords >= 0) & (coords < nx), axis=-1)
        lin = ((batch_ix * NZ + coords[:, 2]) * NXX + coords[:, 0]) * NXY + coords[:, 1]
        nseg = b * NZ * NXX * NXY
        lin = jnp.where(kept, lin, nseg)
        pooled = jax.ops.segment_sum(feats, lin, num_segments=nseg + 1)[:-1]
        out = pooled.reshape(b, NZ, NXX, NXY, c).transpose(0, 1, 4, 2, 3)
        final = out.reshape(b, NZ * c, NXX, NXY)
        return np.asarray(jax.device_get(final))


def plan(coords, x):
    """Build per-core mask/index tables from int voxel coords (vectorized).

    Returns None if the (d,w)/(d,h) factorization doesn't hold (caller then
    uses the host fallback), else a dict of planning tensors.
    """
    cx, cy, cz = coords[..., 0], coords[..., 1], coords[..., 2]
    if not (
        (cx == cx[:, :, :1, :]).all()
        and (cy == cy[:, :, :1, :]).all()
        and (cz == cz[:, :, :, :1]).all()
    ):
        return None

    vx = cx[:, :, 0, :].astype(np.int64)  # (B, D, W)
    vy = cy[:, :, 0, :].astype(np.int64)
    zk = cz[:, :, :, 0] == 0  # (B, D, H) keep mask

    inr = (vx >= 0) & (vx < NXX) & (vy >= 0) & (vy < NXY)
    vox = np.where(inr, vx * NXY + vy, -1)  # (B, D, W)

    # split into the two per-core w-halves: (B, 2, D, WS)
    v = vox.reshape(B, D, 2, WS).transpose(0, 2, 1, 3)

    # scan continuation mask: m=1 iff slot continues the same in-range voxel
    m = np.zeros((B, 2, D, WS), np.float32)
    m[..., 1:] = ((v[..., 1:] == v[..., :-1]) & (v[..., 1:] >= 0)).astype(
        np.float32
    )
    # run-end marker (where the scan state holds the full run total)
    lastw = np.ones((B, 2, D, WS), bool)
    lastw[..., :-1] = v[..., 1:] != v[..., :-1]

    # HW scatter-add races on duplicate indices within a concurrent queue
    # set: a voxel must not receive run totals from two different d-rows of
    # the same core.  (This also guarantees local rows are disjoint across
    # the 4 w-range queues.)  Per-core local row maps follow.
    uvoxes = []
    for b in range(B):
        for hf in range(2):
            real = lastw[b, hf] & (v[b, hf] >= 0)
            vv = v[b, hf][real]
            uv = np.unique(vv)
            if vv.size != uv.size:
                return None
            if uv.size > LSIZE - 1024:  # need trash-row headroom
                return None
            uvoxes.append(uv)

    # int16 scatter indices in LOCAL row space, one table slab per queue,
    # token i = w_local*128 + d within the queue's 11-wide w-range.
    # Discarded tokens spread over the spare rows above the real rows:
    # duplicate-index RMW adds serialize per row on HW, so a single trash
    # row would gate the whole scatter.
    idx = np.zeros((B, 2, NQ, WQ, 128), np.int16)
    for b in range(B):
        for hf in range(2):
            uv = uvoxes[2 * b + hf]
            nv = uv.size
            spread = nv + (
                np.arange(NQ * WQ * 128, dtype=np.int64) % (LSIZE - nv)
            )
            tab = spread.reshape(NQ, WQ, 128).astype(np.int16)
            real = lastw[b, hf] & (v[b, hf] >= 0)  # (D, WS)
            loc = np.zeros((D, WS), np.int64)
            loc[real] = np.searchsorted(uv, v[b, hf][real])
            realT = real.T.reshape(NQ, WQ, D)  # (NQ, WQ, D)
            locT = loc.T.reshape(NQ, WQ, D)
            tab[:, :, :D] = np.where(realT, locT, tab[:, :, :D]).astype(np.int16)
            idx[b, hf] = tab

    # SBUF layout [16, ntok/16]: token i at partition i%16, column i//16;
    # the 16-partition table is then replicated 8x across 128 partitions
    # (one copy per gpsimd Q7 core, per the dma_scatter_add contract)
    idx = idx.reshape(B, 2, NQ, NTOKQ // 16, 16).swapaxes(-1, -2)
    idx = idx.transpose(0, 1, 3, 2, 4).reshape(B, 2, 16, NQ * (NTOKQ // 16))
    idx = np.tile(idx, (1, 1, 8, 1))  # (B, 2, 128, NQ*NTOKQ/16)

    # per-w scan mask (expanded to the (c w) layout on device)
    sm = m  # (B, 2, D, WS)

    # PE h-mask, one 64-wide block per 4-d group. Group g accumulates into
    # PSUM rows [base, base+64) (base = 0 for g<16 else 64); within the block
    # only the group's own d-columns are nonzero:
    #   hm[b, g, 32*j + h, (4*g + j) - base] = zmask[4g+j, h]
    hm = np.zeros((B, GROUPS, 128, 64), np.float32)
    zkf = zk.astype(np.float32)
    for g in range(GROUPS):
        base = 0 if g < 16 else 64
        for j in range(min(4, D - 4 * g)):
            hm[:, g, 32 * j : 32 * j + H, 4 * g + j - base] = zkf[:, 4 * g + j, :]

    # fp8 hybrid: supergroups feeding voxels with many contributions keep
    # bf16 (quantization noise accumulates as sqrt(n) over n summed points).
    # Contribution counts per voxel, then per-supergroup contribution to the
    # hot set; promote supergroups until the predicted noise is comfortable.
    wt = zk.sum(-1)  # (B, D) kept-h count
    promo = set()
    for b in range(B):
        acc = np.zeros(V, np.int64)
        w_pts = np.where(vox[b] >= 0, 1, 0) * wt[b][:, None]
        np.add.at(
            acc,
            np.where(vox[b] >= 0, vox[b], 0),
            np.where(vox[b] >= 0, w_pts, 0),
        )
        hotset = acc > 300
        if not hotset.any():
            continue
        mask_hot = np.where(vox[b] >= 0, hotset[np.where(vox[b] >= 0, vox[b], 0)], False)
        contrib = (mask_hot * w_pts).sum(-1)  # (D,)
        sg_contrib = np.zeros(SG)
        for sg in range(SG):
            sg_contrib[sg] = contrib[8 * sg : min(8 * sg + 8, D)].sum()
        tot = sg_contrib.sum()
        if tot == 0:
            continue
        order = np.argsort(-sg_contrib)
        run = 0.0
        for sg in order:
            if sg_contrib[sg] == 0 or run >= 0.95 * tot:
                break
            promo.add(int(sg))
            run += sg_contrib[sg]
    promo = tuple(sorted(promo))

    return {"hm": hm, "sm": sm, "idx": idx, "uvoxes": uvoxes, "promo": promo}


def build_nc(promo):
    """Build the (single, SPMD) Bass program; `promo` = bf16 supergroups."""
    from concourse import bacc, bass, mybir
    from concourse import tile as tile_mod

    f32 = mybir.dt.float32
    bf16 = mybir.dt.bfloat16
    fp8 = mybir.dt.float8e3
    i16 = mybir.dt.int16

    promo = set(promo)
    n16 = len(promo)
    n8 = SG - n16

    nc = bacc.Bacc(
        trn_type="TRN2",
        target_bir_lowering=False,
        debug=False,
        enable_asserts=False,
        num_devices=N_CORES,
        dynamic_dma_scratch_size=1 << 15,
        num_swdge_queues=NQ,
    )
    # x packed as supergroups of 8 d-slabs: [sg, p=(q h), s*WC + (c w)]
    # (d = 8*sg + 4*s + q; two 4-d halves per DMA tile).  bf16 supergroups
    # and fp8 supergroups live in separate dram tensors.
    x8_d = x16_d = None
    if n8:
        x8_d = nc.dram_tensor("x8", (n8, 128, 2 * WC), fp8, kind="ExternalInput")
    if n16:
        x16_d = nc.dram_tensor(
            "x16", (n16, 128, 2 * WC), bf16, kind="ExternalInput"
        )
    hm8_d = hm16_d = None
    if n8:
        hm8_d = nc.dram_tensor("hm8", (128, 2 * n8 * 64), fp8, kind="ExternalInput")
    if n16:
        hm16_d = nc.dram_tensor(
            "hm16", (128, 2 * n16 * 64), bf16, kind="ExternalInput"
        )
    sm_d = nc.dram_tensor("sm", (D, WS), bf16, kind="ExternalInput")
    idx_d = nc.dram_tensor(
        "idx", (128, NQ * (NTOKQ // 16)), i16, kind="ExternalInput"
    )
    grid = nc.dram_tensor("grid", (LSIZE, 128), bf16, kind="ExternalOutput")

    sems = [nc.alloc_semaphore(f"scat_dma{q}") for q in range(NQ)]

    HC = WC // 2  # c-half split point in the (c w) layout

    with tile_mod.TileContext(nc) as tc:
        with (
            tc.tile_pool(name="const", bufs=1) as cp,
            tc.tile_pool(name="xp", bufs=4) as xp,
            tc.tile_pool(name="yp", bufs=1) as yp,
            tc.tile_pool(name="ps", bufs=1, space="PSUM") as pp,
        ):
            # small tables ride the scalar queue so the first x tile (on the
            # sync queue) starts streaming immediately
            hm8_t = hm16_t = None
            if n8:
                hm8_t = cp.tile([128, 2 * n8 * 64], fp8)
                nc.scalar.dma_start(out=hm8_t[:], in_=hm8_d.ap())
            if n16:
                hm16_t = cp.tile([128, 2 * n16 * 64], bf16)
                nc.scalar.dma_start(out=hm16_t[:], in_=hm16_d.ap())
            sm_s = cp.tile([128, WS], bf16)
            nc.scalar.dma_start(out=sm_s[:D, :], in_=sm_d.ap())
            idx_t = cp.tile([128, NQ * (NTOKQ // 16)], i16)
            nc.scalar.dma_start(out=idx_t[:], in_=idx_d.ap())
            sm_t = cp.tile([128, WC], bf16)

            y_ps = pp.tile([128, WC], f32)  # 7 PSUM banks, (c w) layout
            # scan outputs, one tile per (quadrant, c-half) so the Act/Pool
            # transposes pair with exactly the scan that produced their input
            y_sa0 = yp.tile([128, HC], bf16, tag="ysa0")  # rows 0:64, c-half 0
            y_sa1 = yp.tile([128, WC - HC], bf16, tag="ysa1")
            y_sb0 = yp.tile([128, HC], bf16, tag="ysb0")  # rows 64:118
            y_sb1 = yp.tile([128, WC - HC], bf16, tag="ysb1")
            y_t = yp.tile([128, WC], bf16, tag="yt")  # transposed, (w c)
            # pad partitions feed trash-row tokens; zero them once (engine
            # start-partition must be 32-aligned; rows 96..117 are rewritten
            # by the transpose copy afterwards)
            nc.gpsimd.memset(y_t[96:128, :], 0.0)

            # stream order: bf16 (promoted) supergroups go LAST -- the first
            # DMA wave is then a small fp8 tile, so the PE starts ~8us
            # earlier.  Quadrant completion follows stream order.
            sg_order = [s for s in range(SG) if s not in promo] + sorted(promo)
            g_order = [2 * s + h for s in sg_order for h in (0, 1)]
            qa = [g for g in g_order if g < 16]  # PSUM rows 0:64
            qb = [g for g in g_order if g >= 16]  # PSUM rows 64:118
            sg8_ix = {s: i for i, s in enumerate(x for x in range(SG) if x not in promo)}
            sg16_ix = {s: i for i, s in enumerate(x for x in range(SG) if x in promo)}
            xt = None
            for gi in range(0, len(g_order), 2):
                sg = g_order[gi] // 2
                x_is16 = sg in promo
                dt = bf16 if x_is16 else fp8
                xt = xp.tile([128, 2 * WC], dt, tag="xt")
                # alternate HWDGE queues so per-DMA setup overlaps the
                # previous transfer
                eng = nc.sync if (gi // 2) % 2 == 0 else nc.scalar
                if x_is16:
                    eng.dma_start(out=xt[:], in_=x16_d.ap()[sg16_ix[sg]])
                else:
                    eng.dma_start(out=xt[:], in_=x8_d.ap()[sg8_ix[sg]])
                for half in (0, 1):
                    g = g_order[gi + half]
                    nd = min(4, D - 4 * g)
                    rows = 32 * nd
                    base = 0 if g < 16 else 64
                    mrows = 64 if g < 16 else D - 64
                    first = g in (qa[0], qb[0])
                    last = g in (qa[-1], qb[-1])
                    hm_t = hm16_t if x_is16 else hm8_t
                    hcol = (sg16_ix[sg] if x_is16 else sg8_ix[sg]) * 128 + half * 64
                    for n0 in range(0, WC, 512):
                        nn = min(512, WC - n0)
                        nc.tensor.matmul(
                            out=y_ps[base : base + mrows, n0 : n0 + nn],
                            lhsT=hm_t[:rows, hcol : hcol + mrows],
                            rhs=xt[:rows, half * WC + n0 : half * WC + n0 + nn],
                            start=first,
                            stop=last,
                        )
            # scatter preps emitted right after the matmuls: the Pool queue
            # has nothing else queued, so the ~10us descriptor generation and
            # the per-queue swdge-sem setup run during the x stream (their
            # y_t data read defers to the trigger).  Each queue covers an
            # 11-wide w-range; local rows are disjoint across queues
            # (run-end voxels are unique core-wide), so the four concurrent
            # RMW streams cannot race.
            for q in range(NQ):
                nc.gpsimd.dma_scatter_add(
                    grid.ap()[:, :C],
                    y_t[:, q * WQ * C : (q + 1) * WQ * C].rearrange(
                        "p (w c) -> p w c", c=C
                    ),
                    idx_t[:, q * (NTOKQ // 16) : (q + 1) * (NTOKQ // 16)],
                    NTOKQ,
                    NTOKQ,
                    C,
                    elem_step=128,
                    prepare_only=True,
                    sem=sems[q],
                    queue_num=q,
                )

            # expand the per-w mask to the (c w) layout on-device (ships 80x
            # less table data). Emitted AFTER the matmuls: any earlier and
            # Tile's clock alignment makes the early matmuls wait on the DVE
            # queue (~16-21us PE stall at stream start). Its sm_s data dep
            # still lets it run early, long before the scan needs it.
            nc.vector.tensor_copy(
                out=sm_t[:D].rearrange("p (c w) -> p c w", c=C),
                in_=sm_s[:D, None, :].to_broadcast([D, C, WS]),
            )

            def scan_part(out_t, r0, r1, c0, c1):
                # segmented run-sum along w; chains reset at w=0 of every c
                # (mask is 0 there), so c-column blocks split freely.
                nc.vector.tensor_tensor_scan(
                    out=out_t[r0:r1, : c1 - c0],
                    data0=sm_t[r0:r1, c0:c1],
                    data1=y_ps[r0:r1, c0:c1],
                    initial=0.0,
                    op0=mybir.AluOpType.mult,
                    op1=mybir.AluOpType.add,
                )

            def transpose_part(eng, in_t, r0, r1, c0, c1):
                # (c w) -> (w c) strided copy, pipelined under later scans
                cc0, cc1 = c0 // WS, c1 // WS
                out_ap = y_t[r0:r1].rearrange("p (w c) -> p w c", c=C)[
                    :, :, cc0:cc1
                ]
                in_ap = in_t[r0:r1, : c1 - c0].rearrange("p (c w) -> p w c", w=WS)
                if eng is nc.scalar:
                    eng.copy(out=out_ap, in_=in_ap)
                else:
                    eng.tensor_copy(out=out_ap, in_=in_ap)

            # quadrant B (rows 64:118) finalizes in PSUM first (sg0 streams
            # last), so its scans go first.  Scans split per c-half so each
            # Act transpose runs under the next scan; the Pool queue is left
            # free for the scatter preps/triggers.
            scan_part(y_sb0, 64, D, 0, HC)
            scan_part(y_sb1, 64, D, HC, WC)
            transpose_part(nc.scalar, y_sb0, 64, D, 0, HC)
            scan_part(y_sa0, 0, 64, 0, HC)
            transpose_part(nc.scalar, y_sb1, 64, D, HC, WC)
            scan_part(y_sa1, 0, 64, HC, WC)
            transpose_part(nc.scalar, y_sa0, 0, 64, 0, HC)
            transpose_part(nc.scalar, y_sa1, 0, 64, HC, WC)

            for q in range(NQ):
                nc.gpsimd.trigger_dma(count=None, queue_num=q)

    nc.compile()
    return nc


def make_in_maps(x, p):
    """Per-core input dicts. Core i: batch i//2, w-half i%2."""
    import ml_dtypes

    bf16 = ml_dtypes.bfloat16
    fp8 = ml_dtypes.float8_e3m4
    promo = set(p["promo"])
    x = np.asarray(x)
    in_maps = []
    for core in range(N_CORES):
        b, half = core // 2, core % 2
        # pack [d, h, w, c] -> [sg, (q h), (s c w)] with d = 8 sg + 4 s + q
        blk = x[b, :, :, half * WS : (half + 1) * WS, :]  # (D, H, WS, C)
        blk = np.concatenate(
            [blk, np.zeros((8 * SG - D,) + blk.shape[1:], blk.dtype)], axis=0
        )
        xs = (
            blk.transpose(0, 1, 3, 2)  # (D8, H, C, WS)
            .reshape(SG, 2, 4, H, C, WS)
            .transpose(0, 2, 3, 1, 4, 5)  # (sg, q, h, s, c, w)
            .reshape(SG, 128, 2 * WC)
        )
        sg8 = [s for s in range(SG) if s not in promo]
        sg16 = [s for s in range(SG) if s in promo]
        # hm packed per dtype in stream order: [128, nsg*2*64] with the two
        # 4-d group halves of each supergroup adjacent
        hmb = p["hm"][b]  # (GROUPS, 128, 64) fp32
        hm_sg = np.concatenate(
            [hmb[2 * s : 2 * s + 2].transpose(1, 0, 2).reshape(128, 128) for s in range(SG)],
            axis=1,
        ).reshape(128, SG, 128)
        im = {
            "sm": p["sm"][b, half].astype(bf16, order="C"),
            "idx": np.ascontiguousarray(p["idx"][b, half]),
        }
        if sg8:
            im["x8"] = np.ascontiguousarray(xs[sg8]).astype(fp8, order="C")
            im["hm8"] = np.ascontiguousarray(
                hm_sg[:, sg8].reshape(128, -1)
            ).astype(fp8, order="C")
        if sg16:
            im["x16"] = np.ascontiguousarray(xs[sg16]).astype(bf16, order="C")
            im["hm16"] = np.ascontiguousarray(
                hm_sg[:, sg16].reshape(128, -1)
            ).astype(bf16, order="C")
        in_maps.append(im)
    return in_maps


def assemble(grids, uvoxes):
    """grids: 8 (LSIZE, 128) bf16 arrays in per-core local row space;
    w-half pairs add into the shared (V, C) batch grid."""
    out = np.empty((B, C, NXX, NXY), np.float32)
    for b in range(B):
        s = np.zeros((V, C), np.float32)
        for half in range(2):
            uv = uvoxes[2 * b + half]
            g = np.asarray(grids[2 * b + half])[: uv.size, :C].astype(np.float32)
            s[uv] += g
        out[b] = s.reshape(NXX, NXY, C).transpose(2, 0, 1)
    return out


def _install_ntff_shim():
    """Provide antenv.axon_hooks with an NTFF profile hook driven by ctypes
    into the axon PJRT .so (the agent image's antenv lacks axon_hooks; this
    replicates trn_agent_boot's degraded-away hook). Only used when
    KERNEL_TRACE=1."""
    import contextlib
    import ctypes
    import types

    if "antenv.axon_hooks" in sys.modules:
        return
    so_path = "/opt/axon/libaxon_pjrt.so"
    if not os.path.exists(so_path):
        return
    lib = ctypes.CDLL(so_path)
    if not hasattr(lib, "axon_start_nrt_profile"):
        return
    lib.axon_start_nrt_profile.argtypes = [
        ctypes.POINTER(ctypes.c_int64),
        ctypes.c_size_t,
    ]
    lib.axon_start_nrt_profile.restype = ctypes.c_int64
    lib.axon_stop_nrt_profile.argtypes = [ctypes.c_char_p]
    lib.axon_stop_nrt_profile.restype = ctypes.c_int64

    @contextlib.contextmanager
    def _hook(output_dir, device_ids):
        import jax

        jax.devices()
        if device_ids:
            ids = (ctypes.c_int64 * len(device_ids))(*device_ids)
            rc = lib.axon_start_nrt_profile(ids, len(device_ids))
        else:
            rc = lib.axon_start_nrt_profile(None, 0)
        if rc != 0:
            raise RuntimeError(f"axon_start_nrt_profile rc={rc}")
        try:
            yield
        finally:
            n = lib.axon_stop_nrt_profile(str(output_dir).encode())
            print(f"ntff profile: {n} file(s) written to {output_dir}")

    mod = types.ModuleType("antenv.axon_hooks")
    mod.get_axon_ntff_profile_hook = lambda: _hook
    mod.set_axon_ntff_profile_hook = lambda h: None
    sys.modules["antenv.axon_hooks"] = mod


def kernel(**inputs):
    x = np.asarray(inputs["x"])
    coords = _host_coords(**inputs)
    p = plan(coords, x)
    if p is None:
        return _host_fallback(**inputs)

    key = p["promo"]
    if key not in _NC_CACHE:
        _NC_CACHE[key] = build_nc(key)
    nc = _NC_CACHE[key]

    from concourse.bass_utils import run_bass_kernel_spmd

    trace = bool(int(os.environ.get("KERNEL_TRACE", "0")))
    trace_cores = None
    if trace:
        tc_env = os.environ.get("KERNEL_TRACE_CORES", "0")
        trace_cores = [int(t) for t in tc_env.split(",") if t != ""]
        _install_ntff_shim()
    res = run_bass_kernel_spmd(
        nc,
        make_in_maps(x, p),
        core_ids=list(range(N_CORES)),
        trace=trace,
        trace_cores=trace_cores,
    )
    kernel.last_results = res
    if res.exec_time_ns is not None:
        print(f"HW exec time: {res.exec_time_ns} ns")
    grids = [res.results[i]["grid"] for i in range(N_CORES)]
    return assemble(grids, p["uvoxes"])


kernel.last_results = None
